# revision 24
# baseline (speedup 1.0000x reference)
"""Trainium2 Bass kernel for nn_GCFNN (2-modality GCN+GAT VAE-ish net).

Strategy: row-shard the node dim N=4096 across 8 cores (512 rows each).
Each core holds adj[rows_c].T ([4096, 512], node-j on partitions) resident in
SBUF. Per GCN layer: cores compute their support slice x_c @ W (via streamed
xT tiles), AllGather it to [4096, Hf], then aggregate transposed:
    x1T[f, i] = sum_j s[j, f] * adjT[j, i]   (lhsT = s tiles, rhs = adjT tiles)
so the bias add + leaky-relu fuse into one ScalarE Prelu (bias is
per-partition in feature-major layout). GAT attention is computed with logits
transposed (eT[j, i]) so the masked-exp output pm is directly the lhsT of
att@h; the softmax denominator comes from an extra ones-column matmul.
Attention output, joint product-of-experts, and the 3 predictor MLPs all stay
feature-major; the [15, 512] per-core result is transposed host-side.
"""

import functools
import os
import sys

import numpy as np

if "/opt/trn_rl_repo" not in sys.path:
    sys.path.insert(0, "/opt/trn_rl_repo")

import concourse.bacc as bacc
import concourse.mybir as mybir
import concourse.tile as tile
from concourse.bass_interp import get_hw_module
from concourse.bass_utils import run_bass_kernel_spmd

N, D, H, F2, Z, Y, PH, M = 4096, 400, 256, 128, 64, 5, 128, 2
NCORES = 8
S = N // NCORES          # 512 rows per core
NJT = N // 128           # 32 j-tiles
NIB = S // 128           # 4 i-blocks
DP = 512                 # D padded to 4*128
NDT = DP // 128          # 4 d-tiles
NFB = H // 128           # 2 feature blocks
HA = 130                 # h_aug cols: 128 h + 1 ones + 1 pad
EPS = 1e-8

F32 = mybir.dt.float32
F32R = mybir.dt.float32r
BF16 = mybir.dt.bfloat16

# perf knobs
AGG_FAST = os.environ.get("K_AGG_FAST", "1") == "1"   # float32r big matmuls
ATT_BF16 = os.environ.get("K_ATT_BF16", "1") == "1"   # bf16 attention matmuls

ATT_DT = BF16 if ATT_BF16 else F32
MM_DT = F32R if AGG_FAST else F32
AFT = mybir.ActivationFunctionType
ALU = mybir.AluOpType


def _emit(nc, tc, P):
    """Emit the whole per-core program. P = dict of DRAM param APs."""
    ag1_in, ag1_out, ag2_in, ag2_out = [], [], [], []
    agh_in, agh_out, a2h_in, a2h_out = [], [], [], []
    for m in range(M):
        ag1_in.append(nc.dram_tensor(f"ag1_in{m}", [S, H], MM_DT))
        ag1_out.append(nc.dram_tensor(f"ag1_out{m}", [N, H], MM_DT, addr_space="Shared"))
        ag2_in.append(nc.dram_tensor(f"ag2_in{m}", [S, H], MM_DT))
        ag2_out.append(nc.dram_tensor(f"ag2_out{m}", [N, H], MM_DT, addr_space="Shared"))
        agh_in.append(nc.dram_tensor(f"agh_in{m}", [S, HA], ATT_DT))
        agh_out.append(nc.dram_tensor(f"agh_out{m}", [N, HA], ATT_DT, addr_space="Shared"))
        a2h_in.append(nc.dram_tensor(f"a2h_in{m}", [S, 1], F32))
        a2h_out.append(nc.dram_tensor(f"a2h_out{m}", [N, 1], F32, addr_space="Shared"))

    rg = [list(range(NCORES))]

    with (
        tc.tile_pool(name="persist", bufs=1) as pp,
        tc.tile_pool(name="stream", bufs=5) as sp,
        tc.tile_pool(name="work", bufs=3) as wp,
        tc.tile_pool(name="pmpool", bufs=2) as pmp,
        tc.tile_pool(name="small", bufs=4) as smp,
        tc.tile_pool(name="ps512", bufs=3, space="PSUM") as ps512,
        tc.tile_pool(name="ps256", bufs=2, space="PSUM") as ps256,
        tc.tile_pool(name="pssm", bufs=2, space="PSUM") as pssm,
    ):
        # ---------- persistent loads ----------
        adjT, W1, W2, Wg, b1, b2, ga = [], [], [], [], [], [], []
        for m in range(M):
            t = pp.tile([128, NJT * S], MM_DT, tag=f"adjT{m}", name=f"adjT{m}")
            for j in range(NJT):
                nc.sync.dma_start(
                    out=t[:, j * S:(j + 1) * S],
                    in_=P[f"adjT{m}"][j * 128:(j + 1) * 128, :],
                )
            adjT.append(t)

            t = pp.tile([128, NDT * H], MM_DT, tag=f"W1_{m}", name=f"W1_{m}")
            for k in range(NDT):
                nc.sync.dma_start(
                    out=t[:, k * H:(k + 1) * H],
                    in_=P[f"gc1_W{m}"][k * 128:(k + 1) * 128, :],
                )
            W1.append(t)

            t = pp.tile([128, NFB * H], MM_DT, tag=f"W2_{m}", name=f"W2_{m}")
            for k in range(NFB):
                nc.sync.dma_start(
                    out=t[:, k * H:(k + 1) * H],
                    in_=P[f"gc2_W{m}"][k * 128:(k + 1) * 128, :],
                )
            W2.append(t)

            t = pp.tile([128, NFB * F2], F32, tag=f"Wg_{m}", name=f"Wg_{m}")
            nc.sync.dma_start(
                out=t[:].rearrange("p (t f) -> p t f", t=NFB),
                in_=P[f"gat_W{m}"].rearrange("(t p) f -> p t f", p=128),
            )
            Wg.append(t)

            t = pp.tile([128, NFB], F32, tag=f"b1_{m}", name=f"b1_{m}")
            nc.sync.dma_start(
                out=t[:].rearrange("p (t o) -> p t o", t=NFB),
                in_=P[f"gc1_b{m}"].rearrange("(t p) o -> p t o", p=128),
            )
            b1.append(t)

            t = pp.tile([128, NFB], F32, tag=f"b2_{m}", name=f"b2_{m}")
            nc.sync.dma_start(
                out=t[:].rearrange("p (t o) -> p t o", t=NFB),
                in_=P[f"gc2_b{m}"].rearrange("(t p) o -> p t o", p=128),
            )
            b2.append(t)

            t = pp.tile([128, 2], F32, tag=f"ga_{m}", name=f"ga_{m}")
            nc.sync.dma_start(
                out=t[:].rearrange("p (t o) -> p t o", t=2),
                in_=P[f"gat_a{m}"].rearrange("(t p) o -> p t o", p=128),
            )
            ga.append(t)

        spW1, spb1, spW2, spb2 = [], [], [], []
        for tag in ("0", "1", "j"):
            key = {"0": ("spW1_0", "spb1_0", "spW2_0", "spb2_0"),
                   "1": ("spW1_1", "spb1_1", "spW2_1", "spb2_1"),
                   "j": ("jpW1", "jpb1", "jpW2", "jpb2")}[tag]
            t = pp.tile([Z, PH], F32, tag=f"spW1{tag}", name=f"spW1{tag}")
            nc.sync.dma_start(out=t[:], in_=P[key[0]][:, :])
            spW1.append(t)
            t = pp.tile([PH, 1], F32, tag=f"spb1{tag}", name=f"spb1{tag}")
            nc.sync.dma_start(out=t[:], in_=P[key[1]][:, :])
            spb1.append(t)
            t = pp.tile([PH, Y], F32, tag=f"spW2{tag}", name=f"spW2{tag}")
            nc.sync.dma_start(out=t[:], in_=P[key[2]][:, :])
            spW2.append(t)
            t = pp.tile([Y, 1], F32, tag=f"spb2{tag}", name=f"spb2{tag}")
            nc.sync.dma_start(out=t[:], in_=P[key[3]][:, :])
            spb2.append(t)

        ones_row = pp.tile([1, 128], F32, tag="ones_row")
        nc.vector.memset(ones_row[:], 1.0)
        ones_col = pp.tile([128, 1], F32, tag="ones_col")
        nc.vector.memset(ones_col[:], 1.0)

        # per-modality persistent intermediates
        x1T = [pp.tile([128, NFB * S], MM_DT, tag=f"x1T{m}", name=f"x1T{m}") for m in range(M)]
        x2T = [pp.tile([128, NFB * S], F32, tag=f"x2T{m}", name=f"x2T{m}") for m in range(M)]
        hT = [pp.tile([128, S], F32, tag=f"hT{m}", name=f"hT{m}") for m in range(M)]
        a1hb = [pp.tile([128, S], F32, tag=f"a1hb{m}", name=f"a1hb{m}") for m in range(M)]
        a2hb = [pp.tile([128, NJT], F32, tag=f"a2hb{m}", name=f"a2hb{m}") for m in range(M)]
        attT = [pp.tile([128, S], F32, tag=f"attT{m}", name=f"attT{m}") for m in range(M)]

        # ---------- stage A: support1 = x @ W1 (sharded), AllGather ----------
        def stage_A(m):
            for ib in range(NIB):
                ps = ps256.tile([128, H], F32, tag="psA")
                for k in range(NDT):
                    xt = sp.tile([128, 128], MM_DT, tag="xst", bufs=5)
                    nc.sync.dma_start(
                        out=xt[:],
                        in_=P[f"xT{m}"][k * 128:(k + 1) * 128,
                                        ib * 128:(ib + 1) * 128],
                    )
                    nc.tensor.matmul(
                        ps[:], xt[:], W1[m][:, k * H:(k + 1) * H],
                        start=(k == 0), stop=(k == NDT - 1),
                    )
                sb = wp.tile([128, H], MM_DT, tag="scp")
                nc.vector.tensor_copy(sb[:], ps[:])
                nc.sync.dma_start(out=ag1_in[m][ib * 128:(ib + 1) * 128, :], in_=sb[:])
            nc.gpsimd.collective_compute(
                "AllGather", ALU.bypass, replica_groups=rg,
                ins=[ag1_in[m].ap().opt()], outs=[ag1_out[m].ap().opt()],
            )

        # ---------- aggregation: outT = prelu(adj @ s + b), transposed -------
        def stage_agg(m, ag_out, bias, outT):
            psf = [ps512.tile([128, S], F32, tag="psAgg", name=f"psf{i}")
                   for i in range(NFB)]
            for j in range(NJT):
                st = sp.tile([128, H], MM_DT, tag="sstream", bufs=8)
                nc.sync.dma_start(out=st[:], in_=ag_out[j * 128:(j + 1) * 128, :])
                for fb in range(NFB):
                    nc.tensor.matmul(
                        psf[fb][:],
                        st[:, fb * 128:(fb + 1) * 128],
                        adjT[m][:, j * S:(j + 1) * S],
                        start=(j == 0), stop=(j == NJT - 1),
                    )
            for fb in range(NFB):
                nc.scalar.activation(
                    outT[:, fb * S:(fb + 1) * S], psf[fb][:],
                    AFT.Prelu, bias=bias[:, fb:fb + 1], scale=1.0, alpha=0.25,
                )

        # ---------- stage C: support2 = x1 @ W2 (sharded), AllGather ---------
        def stage_C(m):
            for ib in range(NIB):
                ps = ps256.tile([128, H], F32, tag="psA")
                for fb in range(NFB):
                    nc.tensor.matmul(
                        ps[:],
                        x1T[m][:, fb * S + ib * 128: fb * S + (ib + 1) * 128],
                        W2[m][:, fb * H:(fb + 1) * H],
                        start=(fb == 0), stop=(fb == NFB - 1),
                    )
                sb = wp.tile([128, H], MM_DT, tag="scp")
                nc.vector.tensor_copy(sb[:], ps[:])
                nc.sync.dma_start(out=ag2_in[m][ib * 128:(ib + 1) * 128, :], in_=sb[:])
            nc.gpsimd.collective_compute(
                "AllGather", ALU.bypass, replica_groups=rg,
                ins=[ag2_in[m].ap().opt()], outs=[ag2_out[m].ap().opt()],
            )

        # ---------- stage E: h, hT, a-projections, AllGather h_aug ----------
        def stage_E(m):
            # hT[g, i] = sum_f Wg[f, g] x2T[f, i]
            pshT = ps512.tile([128, S], F32, tag="psAgg")
            for fb in range(NFB):
                nc.tensor.matmul(
                    pshT[:],
                    Wg[m][:, fb * F2:(fb + 1) * F2],
                    x2T[m][:, fb * S:(fb + 1) * S],
                    start=(fb == 0), stop=(fb == NFB - 1),
                )
            nc.vector.tensor_copy(hT[m][:], pshT[:])

            # a1h broadcast [128, S]: lhsT = a1 replicated over free dim
            a1rep = smp.tile([128, 128], F32, tag="sm")
            nc.vector.memset(a1rep[:], 0.0)
            nc.vector.tensor_scalar_add(a1rep[:], a1rep[:], ga[m][:, 0:1])
            psa1 = ps512.tile([128, S], F32, tag="psAgg")
            nc.tensor.matmul(psa1[:], a1rep[:], hT[m][:], start=True, stop=True)
            nc.vector.tensor_copy(a1hb[m][:], psa1[:])

            # a2h per i-block + h blocks -> h_aug bounce
            psa2 = pssm.tile([128, NIB], F32, tag="sm")
            hcat = wp.tile([128, NIB * HA], ATT_DT, tag="hcat", bufs=1)
            for ib in range(NIB):
                nc.tensor.matmul(
                    psa2[:, ib:ib + 1],
                    hT[m][:, ib * 128:(ib + 1) * 128],
                    ga[m][:, 1:2],
                    start=True, stop=True,
                )
                psh = ps256.tile([128, F2], F32, tag="psA")
                for fb in range(NFB):
                    nc.tensor.matmul(
                        psh[:],
                        x2T[m][:, fb * S + ib * 128: fb * S + (ib + 1) * 128],
                        Wg[m][:, fb * F2:(fb + 1) * F2],
                        start=(fb == 0), stop=(fb == NFB - 1),
                    )
                nc.vector.tensor_copy(hcat[:, ib * HA: ib * HA + F2], psh[:])
                nc.vector.memset(hcat[:, ib * HA + F2: ib * HA + F2 + 1], 1.0)
                nc.vector.memset(hcat[:, ib * HA + F2 + 1: ib * HA + HA], 0.0)
            a2sb = wp.tile([128, NIB], F32, tag="a2sb", bufs=1)
            nc.vector.tensor_copy(a2sb[:], psa2[:])
            for ib in range(NIB):
                nc.sync.dma_start(
                    out=agh_in[m][ib * 128:(ib + 1) * 128, :],
                    in_=hcat[:, ib * HA:(ib + 1) * HA],
                )
            nc.sync.dma_start(
                out=a2h_in[m].ap().rearrange("(t p) o -> p (t o)", p=128),
                in_=a2sb[:],
            )
            nc.gpsimd.collective_compute(
                "AllGather", ALU.bypass, replica_groups=rg,
                ins=[agh_in[m].ap().opt()], outs=[agh_out[m].ap().opt()],
            )
            nc.gpsimd.collective_compute(
                "AllGather", ALU.bypass, replica_groups=rg,
                ins=[a2h_in[m].ap().opt()], outs=[a2h_out[m].ap().opt()],
            )

        # ---------- stage F: masked attention, transposed ----------
        def stage_F(m):
            nc.sync.dma_start(
                out=a2hb[m][:],
                in_=a2h_out[m].ap().rearrange("(t p) o -> p (t o)", p=128),
            )
            psO = ps512.tile([128, S], F32, tag="psAgg")
            psden = pssm.tile([1, S], F32, tag="sm")
            for j in range(NJT):
                ht = sp.tile([128, HA], ATT_DT, tag="haugst", bufs=5)
                nc.sync.dma_start(out=ht[:], in_=agh_out[m][j * 128:(j + 1) * 128, :])
                l = wp.tile([128, S], F32, tag="w512")
                nc.scalar.activation(
                    l[:], a1hb[m][:], AFT.Prelu,
                    bias=a2hb[m][:, j:j + 1], scale=1.0, alpha=0.25,
                )
                p = wp.tile([128, S], F32, tag="w512")
                nc.scalar.activation(p[:], l[:], AFT.Exp)
                pm = pmp.tile([128, S], ATT_DT, tag="att_pm", bufs=3)
                nc.vector.scalar_tensor_tensor(
                    out=pm[:], in0=adjT[m][:, j * S:(j + 1) * S].bitcast(F32),
                    scalar=0.0, in1=p[:], op0=ALU.is_gt, op1=ALU.mult,
                )
                nc.tensor.matmul(
                    psO[:], ht[:, 0:F2], pm[:],
                    start=(j == 0), stop=(j == NJT - 1),
                )
                nc.tensor.matmul(
                    psden[:], ht[:, F2:F2 + 1], pm[:],
                    start=(j == 0), stop=(j == NJT - 1),
                )
            rec = smp.tile([1, S], F32, tag="sm")
            nc.vector.reciprocal(rec[:], psden[:])
            psR = ps512.tile([128, S], F32, tag="psAgg")
            nc.tensor.matmul(psR[:], ones_row[:, :], rec[:], start=True, stop=True)
            Rsb = wp.tile([128, S], F32, tag="w512")
            nc.vector.tensor_copy(Rsb[:], psR[:])
            sc = wp.tile([128, S], F32, tag="w512")
            nc.vector.tensor_tensor(out=sc[:], in0=psO[:], in1=Rsb[:], op=ALU.mult)
            nc.scalar.activation(attT[m][:], sc[:], AFT.Prelu, alpha=0.25)

        # ---------- stage G: joint PoE + predictors ----------
        def predictor(zT, which, out_row):
            psa = ps512.tile([128, S], F32, tag="psAgg")
            nc.tensor.matmul(psa[:], spW1[which][:], zT, start=True, stop=True)
            aT = wp.tile([128, S], F32, tag="w512")
            nc.scalar.activation(
                aT[:], psa[:], AFT.Prelu, bias=spb1[which][:, 0:1], scale=1.0,
                alpha=0.25,
            )
            pslg = pssm.tile([Y, S], F32, tag="sm")
            nc.tensor.matmul(pslg[:], spW2[which][:], aT[:], start=True, stop=True)
            ex = smp.tile([Y, S], F32, tag="sm")
            nc.scalar.activation(ex[:], pslg[:], AFT.Exp, bias=spb2[which][:, 0:1], scale=1.0)
            pssum = pssm.tile([1, S], F32, tag="sm")
            nc.tensor.matmul(pssum[:], ones_col[0:Y, 0:1], ex[:], start=True, stop=True)
            rs = smp.tile([1, S], F32, tag="sm")
            nc.vector.reciprocal(rs[:], pssum[:])
            psrb = pssm.tile([Y, S], F32, tag="sm")
            nc.tensor.matmul(psrb[:], ones_row[0:1, 0:Y], rs[:], start=True, stop=True)
            rb = smp.tile([Y, S], F32, tag="sm")
            nc.vector.tensor_copy(rb[:], psrb[:])
            yT = smp.tile([Y, S], F32, tag="sm")
            nc.vector.tensor_tensor(out=yT[:], in0=ex[:], in1=rb[:], op=ALU.mult)
            nc.sync.dma_start(out=P["outT"][out_row:out_row + Y, :], in_=yT[:])

        def stage_G():
            # M0/M1 = broadcast of mask columns to [Z, S] via 0-stride DMA
            Msb = []
            for m in range(M):
                t = smp.tile([Z, S], F32, tag="sm", name=f"Msb{m}")
                nc.sync.dma_start(
                    out=t[:],
                    in_=P["maskT"][0:1, m * S:(m + 1) * S].to_broadcast((Z, S)),
                )
                Msb.append(t)
            pmm = []
            for m in range(M):
                e = smp.tile([Z, S], F32, tag="sm", name=f"poe_e{m}")
                nc.scalar.activation(e[:], attT[m][Z:2 * Z, :], AFT.Exp)
                ep = smp.tile([Z, S], F32, tag="sm", name=f"poe_ep{m}")
                nc.vector.tensor_scalar_add(ep[:], e[:], EPS)
                pr = smp.tile([Z, S], F32, tag="sm", name=f"poe_pr{m}")
                nc.vector.reciprocal(pr[:], ep[:])
                pmt = smp.tile([Z, S], F32, tag="sm", name=f"poe_pm{m}")
                nc.vector.tensor_tensor(out=pmt[:], in0=pr[:], in1=Msb[m][:], op=ALU.mult)
                pmm.append(pmt)
            tmp = smp.tile([Z, S], F32, tag="sm")
            nc.vector.scalar_tensor_tensor(
                out=tmp[:], in0=pmm[0][:], scalar=1.0, in1=pmm[1][:],
                op0=ALU.add, op1=ALU.add,
            )
            nc.vector.tensor_scalar_add(tmp[:], tmp[:], EPS)
            jv = smp.tile([Z, S], F32, tag="sm")
            nc.vector.reciprocal(jv[:], tmp[:])
            n0 = smp.tile([Z, S], F32, tag="sm")
            nc.vector.tensor_tensor(out=n0[:], in0=pmm[0][:], in1=attT[0][0:Z, :], op=ALU.mult)
            n1 = smp.tile([Z, S], F32, tag="sm")
            nc.vector.tensor_tensor(out=n1[:], in0=pmm[1][:], in1=attT[1][0:Z, :], op=ALU.mult)
            nsum = smp.tile([Z, S], F32, tag="sm")
            nc.vector.tensor_tensor(out=nsum[:], in0=n0[:], in1=n1[:], op=ALU.add)
            jmu = smp.tile([Z, S], F32, tag="sm")
            nc.vector.tensor_tensor(out=jmu[:], in0=jv[:], in1=nsum[:], op=ALU.mult)

            predictor(jmu[:], 2, 0)
            predictor(attT[0][0:Z, :], 0, Y)
            predictor(attT[1][0:Z, :], 1, 2 * Y)

        # ---------- emission order (interleave modalities for overlap) ----
        # K_REPS>1 repeats the body for marginal-cost timing (bench only).
        for _ in range(int(os.environ.get("K_REPS", "1"))):
            stage_A(0)
            stage_A(1)
            stage_agg(0, ag1_out[0], b1[0], x1T[0])
            stage_C(0)
            stage_agg(1, ag1_out[1], b1[1], x1T[1])
            stage_C(1)
            stage_agg(0, ag2_out[0], b2[0], x2T[0])
            stage_E(0)
            stage_F(0)
            stage_agg(1, ag2_out[1], b2[1], x2T[1])
            stage_E(1)
            stage_F(1)
            stage_G()


@functools.lru_cache(maxsize=1)
def _get_compiled():
    nc = bacc.Bacc("TRN2", target_bir_lowering=False, debug=False,
                   num_devices=NCORES)
    P = {}
    for m in range(M):
        P[f"adjT{m}"] = nc.dram_tensor(f"adjT{m}", [N, S], MM_DT, kind="ExternalInput").ap()
        P[f"xT{m}"] = nc.dram_tensor(f"xT{m}", [DP, S], MM_DT, kind="ExternalInput").ap()
        P[f"gc1_W{m}"] = nc.dram_tensor(f"gc1_W{m}", [DP, H], MM_DT, kind="ExternalInput").ap()
        P[f"gc1_b{m}"] = nc.dram_tensor(f"gc1_b{m}", [H, 1], F32, kind="ExternalInput").ap()
        P[f"gc2_W{m}"] = nc.dram_tensor(f"gc2_W{m}", [H, H], MM_DT, kind="ExternalInput").ap()
        P[f"gc2_b{m}"] = nc.dram_tensor(f"gc2_b{m}", [H, 1], F32, kind="ExternalInput").ap()
        P[f"gat_W{m}"] = nc.dram_tensor(f"gat_W{m}", [H, F2], F32, kind="ExternalInput").ap()
        P[f"gat_a{m}"] = nc.dram_tensor(f"gat_a{m}", [2 * F2, 1], F32, kind="ExternalInput").ap()
        P[f"spW1_{m}"] = nc.dram_tensor(f"spW1_{m}", [Z, PH], F32, kind="ExternalInput").ap()
        P[f"spb1_{m}"] = nc.dram_tensor(f"spb1_{m}", [PH, 1], F32, kind="ExternalInput").ap()
        P[f"spW2_{m}"] = nc.dram_tensor(f"spW2_{m}", [PH, Y], F32, kind="ExternalInput").ap()
        P[f"spb2_{m}"] = nc.dram_tensor(f"spb2_{m}", [Y, 1], F32, kind="ExternalInput").ap()
    P["jpW1"] = nc.dram_tensor("jpW1", [Z, PH], F32, kind="ExternalInput").ap()
    P["jpb1"] = nc.dram_tensor("jpb1", [PH, 1], F32, kind="ExternalInput").ap()
    P["jpW2"] = nc.dram_tensor("jpW2", [PH, Y], F32, kind="ExternalInput").ap()
    P["jpb2"] = nc.dram_tensor("jpb2", [Y, 1], F32, kind="ExternalInput").ap()
    P["maskT"] = nc.dram_tensor("maskT", [1, M * S], F32, kind="ExternalInput").ap()
    P["outT"] = nc.dram_tensor("outT", [3 * Y, S], F32, kind="ExternalOutput").ap()

    with tile.TileContext(nc) as tc:
        _emit(nc, tc, P)
    nc.compile()
    nc.m = get_hw_module(nc.m)
    return nc


def _round_f32r(a):
    """Round fp32 array to fp32r (matmul operand precision) if AGG_FAST."""
    return a


def _shard_inputs(inputs):
    f = np.float32
    in_maps = []
    pad_w = []
    for m in range(M):
        w = np.zeros((DP, H), f)
        w[:D, :] = inputs[f"gc1_W{m}"]
        pad_w.append(np.ascontiguousarray(w))
    for c in range(NCORES):
        r0, r1 = c * S, (c + 1) * S
        im = {}
        for m in range(M):
            im[f"adjT{m}"] = np.ascontiguousarray(
                np.asarray(inputs[f"adj{m}"], f)[r0:r1, :].T)
            xp = np.zeros((DP, S), f)
            xp[:D, :] = np.asarray(inputs[f"x{m}"], f)[r0:r1, :].T
            im[f"xT{m}"] = xp
            im[f"gc1_W{m}"] = pad_w[m]
            im[f"gc1_b{m}"] = np.asarray(inputs[f"gc1_b{m}"], f).reshape(H, 1)
            im[f"gc2_W{m}"] = np.ascontiguousarray(np.asarray(inputs[f"gc2_W{m}"], f))
            im[f"gc2_b{m}"] = np.asarray(inputs[f"gc2_b{m}"], f).reshape(H, 1)
            im[f"gat_W{m}"] = np.ascontiguousarray(np.asarray(inputs[f"gat_W{m}"], f))
            im[f"gat_a{m}"] = np.ascontiguousarray(np.asarray(inputs[f"gat_a{m}"], f))
            im[f"spW1_{m}"] = np.ascontiguousarray(np.asarray(inputs[f"spW1_{m}"], f))
            im[f"spb1_{m}"] = np.asarray(inputs[f"spb1_{m}"], f).reshape(PH, 1)
            im[f"spW2_{m}"] = np.ascontiguousarray(np.asarray(inputs[f"spW2_{m}"], f))
            im[f"spb2_{m}"] = np.asarray(inputs[f"spb2_{m}"], f).reshape(Y, 1)
        im["jpW1"] = np.ascontiguousarray(np.asarray(inputs["jpW1"], f))
        im["jpb1"] = np.asarray(inputs["jpb1"], f).reshape(PH, 1)
        im["jpW2"] = np.ascontiguousarray(np.asarray(inputs["jpW2"], f))
        im["jpb2"] = np.asarray(inputs["jpb2"], f).reshape(Y, 1)
        im["maskT"] = np.ascontiguousarray(
            np.asarray(inputs["mask"], f)[r0:r1, :].T.reshape(1, M * S))
        in_maps.append(im)
    return in_maps


def run(inputs, trace=False):
    nc = _get_compiled()
    in_maps = _shard_inputs(inputs)
    res = run_bass_kernel_spmd(nc, in_maps, list(range(NCORES)), trace=trace)
    out = np.zeros((N, 3 * Y), np.float32)
    for c in range(NCORES):
        out[c * S:(c + 1) * S, :] = res.results[c]["outT"].T
    return out, res


def kernel(**inputs):
    out, _ = run(inputs)
    return out


# revision 25
# speedup vs baseline: 3.1071x; 3.1071x over previous
"""Trainium2 Bass kernel for nn_GCFNN (2-modality GCN+GAT VAE-ish net).

Strategy: row-shard the node dim N=4096 across 8 cores (512 rows each).
Each core holds adj[rows_c].T ([4096, 512], node-j on partitions) resident in
SBUF. Per GCN layer: cores compute their support slice x_c @ W (via streamed
xT tiles), AllGather it to [4096, Hf], then aggregate transposed:
    x1T[f, i] = sum_j s[j, f] * adjT[j, i]   (lhsT = s tiles, rhs = adjT tiles)
so the bias add + leaky-relu fuse into one ScalarE Prelu (bias is
per-partition in feature-major layout). GAT attention is computed with logits
transposed (eT[j, i]) so the masked-exp output pm is directly the lhsT of
att@h; the softmax denominator comes from an extra ones-column matmul.
Attention output, joint product-of-experts, and the 3 predictor MLPs all stay
feature-major; the [15, 512] per-core result is transposed host-side.
"""

import functools
import os
import sys

import numpy as np

if "/opt/trn_rl_repo" not in sys.path:
    sys.path.insert(0, "/opt/trn_rl_repo")

import concourse.bacc as bacc
import concourse.mybir as mybir
import concourse.tile as tile
from concourse.bass_interp import get_hw_module
from concourse.bass_utils import run_bass_kernel_spmd

N, D, H, F2, Z, Y, PH, M = 4096, 400, 256, 128, 64, 5, 128, 2
NCORES = 8
S = N // NCORES          # 512 rows per core
NJT = N // 128           # 32 j-tiles
NIB = S // 128           # 4 i-blocks
DP = 512                 # D padded to 4*128
NDT = DP // 128          # 4 d-tiles
NFB = H // 128           # 2 feature blocks
HA = 130                 # h_aug cols: 128 h + 1 ones + 1 pad
EPS = 1e-8

F32 = mybir.dt.float32
F32R = mybir.dt.float32r
BF16 = mybir.dt.bfloat16

# perf knobs
AGG_FAST = os.environ.get("K_AGG_FAST", "1") == "1"   # float32r big matmuls
ATT_BF16 = os.environ.get("K_ATT_BF16", "1") == "1"   # bf16 attention matmuls

ATT_DT = BF16 if ATT_BF16 else F32
MM_DT = F32R if AGG_FAST else F32
AFT = mybir.ActivationFunctionType
ALU = mybir.AluOpType


def _emit(nc, tc, P):
    """Emit the whole per-core program. P = dict of DRAM param APs."""
    ag1_in, ag1_out, ag2_in, ag2_out = [], [], [], []
    agh_in, agh_out, a2h_in, a2h_out = [], [], [], []
    for m in range(M):
        ag1_in.append(nc.dram_tensor(f"ag1_in{m}", [S, H], MM_DT))
        ag1_out.append(nc.dram_tensor(f"ag1_out{m}", [N, H], MM_DT, addr_space="Shared"))
        ag2_in.append(nc.dram_tensor(f"ag2_in{m}", [S, H], MM_DT))
        ag2_out.append(nc.dram_tensor(f"ag2_out{m}", [N, H], MM_DT, addr_space="Shared"))
        agh_in.append(nc.dram_tensor(f"agh_in{m}", [S, HA], ATT_DT))
        agh_out.append(nc.dram_tensor(f"agh_out{m}", [N, HA], ATT_DT, addr_space="Shared"))
        a2h_in.append(nc.dram_tensor(f"a2h_in{m}", [S, 1], F32))
        a2h_out.append(nc.dram_tensor(f"a2h_out{m}", [N, 1], F32, addr_space="Shared"))

    rg = [list(range(NCORES))]

    with (
        tc.tile_pool(name="persist", bufs=1) as pp,
        tc.tile_pool(name="stream", bufs=5) as sp,
        tc.tile_pool(name="work", bufs=3) as wp,
        tc.tile_pool(name="pmpool", bufs=2) as pmp,
        tc.tile_pool(name="small", bufs=4) as smp,
        tc.tile_pool(name="ps512", bufs=3, space="PSUM") as ps512,
        tc.tile_pool(name="ps256", bufs=2, space="PSUM") as ps256,
        tc.tile_pool(name="pssm", bufs=2, space="PSUM") as pssm,
    ):
        # ---------- persistent loads ----------
        adjT, W1, W2, Wg, b1, b2, ga = [], [], [], [], [], [], []
        for m in range(M):
            t = pp.tile([128, NJT * S], MM_DT, tag=f"adjT{m}", name=f"adjT{m}")
            for j in range(NJT):
                nc.sync.dma_start(
                    out=t[:, j * S:(j + 1) * S],
                    in_=P[f"adjT{m}"][j * 128:(j + 1) * 128, :],
                )
            adjT.append(t)

            t = pp.tile([128, NDT * H], MM_DT, tag=f"W1_{m}", name=f"W1_{m}")
            for k in range(NDT):
                nc.sync.dma_start(
                    out=t[:, k * H:(k + 1) * H],
                    in_=P[f"gc1_W{m}"][k * 128:(k + 1) * 128, :],
                )
            W1.append(t)

            t = pp.tile([128, NFB * H], MM_DT, tag=f"W2_{m}", name=f"W2_{m}")
            for k in range(NFB):
                nc.sync.dma_start(
                    out=t[:, k * H:(k + 1) * H],
                    in_=P[f"gc2_W{m}"][k * 128:(k + 1) * 128, :],
                )
            W2.append(t)

            t = pp.tile([128, NFB * F2], F32, tag=f"Wg_{m}", name=f"Wg_{m}")
            nc.sync.dma_start(
                out=t[:].rearrange("p (t f) -> p t f", t=NFB),
                in_=P[f"gat_W{m}"].rearrange("(t p) f -> p t f", p=128),
            )
            Wg.append(t)

            t = pp.tile([128, NFB], F32, tag=f"b1_{m}", name=f"b1_{m}")
            nc.sync.dma_start(
                out=t[:].rearrange("p (t o) -> p t o", t=NFB),
                in_=P[f"gc1_b{m}"].rearrange("(t p) o -> p t o", p=128),
            )
            b1.append(t)

            t = pp.tile([128, NFB], F32, tag=f"b2_{m}", name=f"b2_{m}")
            nc.sync.dma_start(
                out=t[:].rearrange("p (t o) -> p t o", t=NFB),
                in_=P[f"gc2_b{m}"].rearrange("(t p) o -> p t o", p=128),
            )
            b2.append(t)

            t = pp.tile([128, 2], F32, tag=f"ga_{m}", name=f"ga_{m}")
            nc.sync.dma_start(
                out=t[:].rearrange("p (t o) -> p t o", t=2),
                in_=P[f"gat_a{m}"].rearrange("(t p) o -> p t o", p=128),
            )
            ga.append(t)

        spW1, spb1, spW2, spb2 = [], [], [], []
        for tag in ("0", "1", "j"):
            key = {"0": ("spW1_0", "spb1_0", "spW2_0", "spb2_0"),
                   "1": ("spW1_1", "spb1_1", "spW2_1", "spb2_1"),
                   "j": ("jpW1", "jpb1", "jpW2", "jpb2")}[tag]
            t = pp.tile([Z, PH], F32, tag=f"spW1{tag}", name=f"spW1{tag}")
            nc.sync.dma_start(out=t[:], in_=P[key[0]][:, :])
            spW1.append(t)
            t = pp.tile([PH, 1], F32, tag=f"spb1{tag}", name=f"spb1{tag}")
            nc.sync.dma_start(out=t[:], in_=P[key[1]][:, :])
            spb1.append(t)
            t = pp.tile([PH, Y], F32, tag=f"spW2{tag}", name=f"spW2{tag}")
            nc.sync.dma_start(out=t[:], in_=P[key[2]][:, :])
            spW2.append(t)
            t = pp.tile([Y, 1], F32, tag=f"spb2{tag}", name=f"spb2{tag}")
            nc.sync.dma_start(out=t[:], in_=P[key[3]][:, :])
            spb2.append(t)

        ones_row = pp.tile([1, 128], F32, tag="ones_row")
        nc.vector.memset(ones_row[:], 1.0)
        ones_col = pp.tile([128, 1], F32, tag="ones_col")
        nc.vector.memset(ones_col[:], 1.0)

        # per-modality persistent intermediates
        x1T = [pp.tile([128, NFB * S], MM_DT, tag=f"x1T{m}", name=f"x1T{m}") for m in range(M)]
        x2T = [pp.tile([128, NFB * S], F32, tag=f"x2T{m}", name=f"x2T{m}") for m in range(M)]
        hT = [pp.tile([128, S], F32, tag=f"hT{m}", name=f"hT{m}") for m in range(M)]
        a1hb = [pp.tile([128, S], F32, tag=f"a1hb{m}", name=f"a1hb{m}") for m in range(M)]
        a2hb = [pp.tile([128, NJT], F32, tag=f"a2hb{m}", name=f"a2hb{m}") for m in range(M)]
        attT = [pp.tile([128, S], F32, tag=f"attT{m}", name=f"attT{m}") for m in range(M)]

        # ---------- stage A: support1 = x @ W1 (sharded), AllGather ----------
        def stage_A(m):
            for ib in range(NIB):
                ps = ps256.tile([128, H], F32, tag="psA")
                for k in range(NDT):
                    xt = sp.tile([128, 128], MM_DT, tag="xst", bufs=5)
                    nc.sync.dma_start(
                        out=xt[:],
                        in_=P[f"xT{m}"][k * 128:(k + 1) * 128,
                                        ib * 128:(ib + 1) * 128],
                    )
                    nc.tensor.matmul(
                        ps[:], xt[:], W1[m][:, k * H:(k + 1) * H],
                        start=(k == 0), stop=(k == NDT - 1),
                    )
                sb = wp.tile([128, H], MM_DT, tag="scp")
                nc.vector.tensor_copy(sb[:], ps[:])
                nc.sync.dma_start(out=ag1_in[m][ib * 128:(ib + 1) * 128, :], in_=sb[:])
            nc.gpsimd.collective_compute(
                "AllGather", ALU.bypass, replica_groups=rg,
                ins=[ag1_in[m].ap().opt()], outs=[ag1_out[m].ap().opt()],
            )

        # ---------- aggregation: outT = prelu(adj @ s + b), transposed -------
        def stage_agg(m, ag_out, bias, outT):
            psf = [ps512.tile([128, S], F32, tag="psAgg", name=f"psf{i}")
                   for i in range(NFB)]
            for j in range(NJT):
                st = sp.tile([128, H], MM_DT, tag="sstream", bufs=8)
                nc.sync.dma_start(out=st[:], in_=ag_out[j * 128:(j + 1) * 128, :])
                for fb in range(NFB):
                    nc.tensor.matmul(
                        psf[fb][:],
                        st[:, fb * 128:(fb + 1) * 128],
                        adjT[m][:, j * S:(j + 1) * S],
                        start=(j == 0), stop=(j == NJT - 1),
                    )
            for fb in range(NFB):
                nc.scalar.activation(
                    outT[:, fb * S:(fb + 1) * S], psf[fb][:],
                    AFT.Prelu, bias=bias[:, fb:fb + 1], scale=1.0, alpha=0.25,
                )

        # ---------- stage C: support2 = x1 @ W2 (sharded), AllGather ---------
        def stage_C(m):
            for ib in range(NIB):
                ps = ps256.tile([128, H], F32, tag="psA")
                for fb in range(NFB):
                    nc.tensor.matmul(
                        ps[:],
                        x1T[m][:, fb * S + ib * 128: fb * S + (ib + 1) * 128],
                        W2[m][:, fb * H:(fb + 1) * H],
                        start=(fb == 0), stop=(fb == NFB - 1),
                    )
                sb = wp.tile([128, H], MM_DT, tag="scp")
                nc.vector.tensor_copy(sb[:], ps[:])
                nc.sync.dma_start(out=ag2_in[m][ib * 128:(ib + 1) * 128, :], in_=sb[:])
            nc.gpsimd.collective_compute(
                "AllGather", ALU.bypass, replica_groups=rg,
                ins=[ag2_in[m].ap().opt()], outs=[ag2_out[m].ap().opt()],
            )

        # ---------- stage E: h, hT, a-projections, AllGather h_aug ----------
        def stage_E(m):
            # hT[g, i] = sum_f Wg[f, g] x2T[f, i]
            pshT = ps512.tile([128, S], F32, tag="psAgg")
            for fb in range(NFB):
                nc.tensor.matmul(
                    pshT[:],
                    Wg[m][:, fb * F2:(fb + 1) * F2],
                    x2T[m][:, fb * S:(fb + 1) * S],
                    start=(fb == 0), stop=(fb == NFB - 1),
                )
            nc.vector.tensor_copy(hT[m][:], pshT[:])

            # a1h broadcast [128, S]: lhsT = a1 replicated over free dim
            a1rep = smp.tile([128, 128], F32, tag="sm")
            nc.vector.memset(a1rep[:], 0.0)
            nc.vector.tensor_scalar_add(a1rep[:], a1rep[:], ga[m][:, 0:1])
            psa1 = ps512.tile([128, S], F32, tag="psAgg")
            nc.tensor.matmul(psa1[:], a1rep[:], hT[m][:], start=True, stop=True)
            nc.vector.tensor_copy(a1hb[m][:], psa1[:])

            # a2h per i-block + h blocks -> h_aug bounce
            psa2 = pssm.tile([128, NIB], F32, tag="sm")
            hcat = wp.tile([128, NIB * HA], ATT_DT, tag="hcat", bufs=1)
            for ib in range(NIB):
                nc.tensor.matmul(
                    psa2[:, ib:ib + 1],
                    hT[m][:, ib * 128:(ib + 1) * 128],
                    ga[m][:, 1:2],
                    start=True, stop=True,
                )
                psh = ps256.tile([128, F2], F32, tag="psA")
                for fb in range(NFB):
                    nc.tensor.matmul(
                        psh[:],
                        x2T[m][:, fb * S + ib * 128: fb * S + (ib + 1) * 128],
                        Wg[m][:, fb * F2:(fb + 1) * F2],
                        start=(fb == 0), stop=(fb == NFB - 1),
                    )
                nc.vector.tensor_copy(hcat[:, ib * HA: ib * HA + F2], psh[:])
                nc.vector.memset(hcat[:, ib * HA + F2: ib * HA + F2 + 1], 1.0)
                nc.vector.memset(hcat[:, ib * HA + F2 + 1: ib * HA + HA], 0.0)
            a2sb = wp.tile([128, NIB], F32, tag="a2sb", bufs=1)
            nc.vector.tensor_copy(a2sb[:], psa2[:])
            for ib in range(NIB):
                nc.sync.dma_start(
                    out=agh_in[m][ib * 128:(ib + 1) * 128, :],
                    in_=hcat[:, ib * HA:(ib + 1) * HA],
                )
            nc.sync.dma_start(
                out=a2h_in[m].ap().rearrange("(t p) o -> p (t o)", p=128),
                in_=a2sb[:],
            )
            nc.gpsimd.collective_compute(
                "AllGather", ALU.bypass, replica_groups=rg,
                ins=[agh_in[m].ap().opt()], outs=[agh_out[m].ap().opt()],
            )
            nc.gpsimd.collective_compute(
                "AllGather", ALU.bypass, replica_groups=rg,
                ins=[a2h_in[m].ap().opt()], outs=[a2h_out[m].ap().opt()],
            )

        # ---------- stage F: masked attention, transposed ----------
        def stage_F(m):
            nc.sync.dma_start(
                out=a2hb[m][:],
                in_=a2h_out[m].ap().rearrange("(t p) o -> p (t o)", p=128),
            )
            psO = ps512.tile([128, S], F32, tag="psAgg")
            psden = pssm.tile([1, S], F32, tag="sm")
            for j in range(NJT):
                ht = sp.tile([128, HA], ATT_DT, tag="haugst", bufs=5)
                nc.sync.dma_start(out=ht[:], in_=agh_out[m][j * 128:(j + 1) * 128, :])
                l = wp.tile([128, S], F32, tag="w512")
                nc.scalar.activation(
                    l[:], a1hb[m][:], AFT.Prelu,
                    bias=a2hb[m][:, j:j + 1], scale=1.0, alpha=0.25,
                )
                p = wp.tile([128, S], F32, tag="w512")
                nc.scalar.activation(p[:], l[:], AFT.Exp)
                pm = pmp.tile([128, S], ATT_DT, tag="att_pm", bufs=3)
                nc.vector.scalar_tensor_tensor(
                    out=pm[:], in0=adjT[m][:, j * S:(j + 1) * S].bitcast(F32),
                    scalar=0.0, in1=p[:], op0=ALU.is_gt, op1=ALU.mult,
                )
                nc.tensor.matmul(
                    psO[:], ht[:, 0:F2], pm[:],
                    start=(j == 0), stop=(j == NJT - 1),
                )
                nc.tensor.matmul(
                    psden[:], ht[:, F2:F2 + 1], pm[:],
                    start=(j == 0), stop=(j == NJT - 1),
                )
            rec = smp.tile([1, S], F32, tag="sm")
            nc.vector.reciprocal(rec[:], psden[:])
            psR = ps512.tile([128, S], F32, tag="psAgg")
            nc.tensor.matmul(psR[:], ones_row[:, :], rec[:], start=True, stop=True)
            Rsb = wp.tile([128, S], F32, tag="w512")
            nc.vector.tensor_copy(Rsb[:], psR[:])
            sc = wp.tile([128, S], F32, tag="w512")
            nc.vector.tensor_tensor(out=sc[:], in0=psO[:], in1=Rsb[:], op=ALU.mult)
            nc.scalar.activation(attT[m][:], sc[:], AFT.Prelu, alpha=0.25)

        # ---------- stage G: joint PoE + predictors ----------
        def predictor(zT, which, out_row):
            psa = ps512.tile([128, S], F32, tag="psAgg")
            nc.tensor.matmul(psa[:], spW1[which][:], zT, start=True, stop=True)
            aT = wp.tile([128, S], F32, tag="w512")
            nc.scalar.activation(
                aT[:], psa[:], AFT.Prelu, bias=spb1[which][:, 0:1], scale=1.0,
                alpha=0.25,
            )
            pslg = pssm.tile([Y, S], F32, tag="sm")
            nc.tensor.matmul(pslg[:], spW2[which][:], aT[:], start=True, stop=True)
            ex = smp.tile([Y, S], F32, tag="sm")
            nc.scalar.activation(ex[:], pslg[:], AFT.Exp, bias=spb2[which][:, 0:1], scale=1.0)
            pssum = pssm.tile([1, S], F32, tag="sm")
            nc.tensor.matmul(pssum[:], ones_col[0:Y, 0:1], ex[:], start=True, stop=True)
            rs = smp.tile([1, S], F32, tag="sm")
            nc.vector.reciprocal(rs[:], pssum[:])
            psrb = pssm.tile([Y, S], F32, tag="sm")
            nc.tensor.matmul(psrb[:], ones_row[0:1, 0:Y], rs[:], start=True, stop=True)
            rb = smp.tile([Y, S], F32, tag="sm")
            nc.vector.tensor_copy(rb[:], psrb[:])
            yT = smp.tile([Y, S], F32, tag="sm")
            nc.vector.tensor_tensor(out=yT[:], in0=ex[:], in1=rb[:], op=ALU.mult)
            nc.sync.dma_start(out=P["outT"][out_row:out_row + Y, :], in_=yT[:])

        def stage_G():
            # M0/M1 = broadcast of mask columns to [Z, S] via 0-stride DMA
            Msb = []
            for m in range(M):
                t = smp.tile([Z, S], F32, tag="sm", name=f"Msb{m}")
                nc.sync.dma_start(
                    out=t[:],
                    in_=P["maskT"][0:1, m * S:(m + 1) * S].to_broadcast((Z, S)),
                )
                Msb.append(t)
            pmm = []
            for m in range(M):
                e = smp.tile([Z, S], F32, tag="sm", name=f"poe_e{m}")
                nc.scalar.activation(e[:], attT[m][Z:2 * Z, :], AFT.Exp)
                ep = smp.tile([Z, S], F32, tag="sm", name=f"poe_ep{m}")
                nc.vector.tensor_scalar_add(ep[:], e[:], EPS)
                pr = smp.tile([Z, S], F32, tag="sm", name=f"poe_pr{m}")
                nc.vector.reciprocal(pr[:], ep[:])
                pmt = smp.tile([Z, S], F32, tag="sm", name=f"poe_pm{m}")
                nc.vector.tensor_tensor(out=pmt[:], in0=pr[:], in1=Msb[m][:], op=ALU.mult)
                pmm.append(pmt)
            tmp = smp.tile([Z, S], F32, tag="sm")
            nc.vector.scalar_tensor_tensor(
                out=tmp[:], in0=pmm[0][:], scalar=1.0, in1=pmm[1][:],
                op0=ALU.add, op1=ALU.add,
            )
            nc.vector.tensor_scalar_add(tmp[:], tmp[:], EPS)
            jv = smp.tile([Z, S], F32, tag="sm")
            nc.vector.reciprocal(jv[:], tmp[:])
            n0 = smp.tile([Z, S], F32, tag="sm")
            nc.vector.tensor_tensor(out=n0[:], in0=pmm[0][:], in1=attT[0][0:Z, :], op=ALU.mult)
            n1 = smp.tile([Z, S], F32, tag="sm")
            nc.vector.tensor_tensor(out=n1[:], in0=pmm[1][:], in1=attT[1][0:Z, :], op=ALU.mult)
            nsum = smp.tile([Z, S], F32, tag="sm")
            nc.vector.tensor_tensor(out=nsum[:], in0=n0[:], in1=n1[:], op=ALU.add)
            jmu = smp.tile([Z, S], F32, tag="sm")
            nc.vector.tensor_tensor(out=jmu[:], in0=jv[:], in1=nsum[:], op=ALU.mult)

            predictor(jmu[:], 2, 0)

        # ---------- emission order (interleave modalities for overlap) ----
        # K_REPS>1 repeats the body for marginal-cost timing (bench only).
        for _ in range(int(os.environ.get("K_REPS", "1"))):
            stage_A(0)
            stage_A(1)
            stage_agg(0, ag1_out[0], b1[0], x1T[0])
            stage_C(0)
            stage_agg(1, ag1_out[1], b1[1], x1T[1])
            stage_C(1)
            stage_agg(0, ag2_out[0], b2[0], x2T[0])
            stage_E(0)
            stage_F(0)
            predictor(attT[0][0:Z, :], 0, Y)
            stage_agg(1, ag2_out[1], b2[1], x2T[1])
            stage_E(1)
            stage_F(1)
            predictor(attT[1][0:Z, :], 1, 2 * Y)
            stage_G()


@functools.lru_cache(maxsize=1)
def _get_compiled():
    nc = bacc.Bacc("TRN2", target_bir_lowering=False, debug=False,
                   num_devices=NCORES)
    P = {}
    for m in range(M):
        P[f"adjT{m}"] = nc.dram_tensor(f"adjT{m}", [N, S], MM_DT, kind="ExternalInput").ap()
        P[f"xT{m}"] = nc.dram_tensor(f"xT{m}", [DP, S], MM_DT, kind="ExternalInput").ap()
        P[f"gc1_W{m}"] = nc.dram_tensor(f"gc1_W{m}", [DP, H], MM_DT, kind="ExternalInput").ap()
        P[f"gc1_b{m}"] = nc.dram_tensor(f"gc1_b{m}", [H, 1], F32, kind="ExternalInput").ap()
        P[f"gc2_W{m}"] = nc.dram_tensor(f"gc2_W{m}", [H, H], MM_DT, kind="ExternalInput").ap()
        P[f"gc2_b{m}"] = nc.dram_tensor(f"gc2_b{m}", [H, 1], F32, kind="ExternalInput").ap()
        P[f"gat_W{m}"] = nc.dram_tensor(f"gat_W{m}", [H, F2], F32, kind="ExternalInput").ap()
        P[f"gat_a{m}"] = nc.dram_tensor(f"gat_a{m}", [2 * F2, 1], F32, kind="ExternalInput").ap()
        P[f"spW1_{m}"] = nc.dram_tensor(f"spW1_{m}", [Z, PH], F32, kind="ExternalInput").ap()
        P[f"spb1_{m}"] = nc.dram_tensor(f"spb1_{m}", [PH, 1], F32, kind="ExternalInput").ap()
        P[f"spW2_{m}"] = nc.dram_tensor(f"spW2_{m}", [PH, Y], F32, kind="ExternalInput").ap()
        P[f"spb2_{m}"] = nc.dram_tensor(f"spb2_{m}", [Y, 1], F32, kind="ExternalInput").ap()
    P["jpW1"] = nc.dram_tensor("jpW1", [Z, PH], F32, kind="ExternalInput").ap()
    P["jpb1"] = nc.dram_tensor("jpb1", [PH, 1], F32, kind="ExternalInput").ap()
    P["jpW2"] = nc.dram_tensor("jpW2", [PH, Y], F32, kind="ExternalInput").ap()
    P["jpb2"] = nc.dram_tensor("jpb2", [Y, 1], F32, kind="ExternalInput").ap()
    P["maskT"] = nc.dram_tensor("maskT", [1, M * S], F32, kind="ExternalInput").ap()
    P["outT"] = nc.dram_tensor("outT", [3 * Y, S], F32, kind="ExternalOutput").ap()

    with tile.TileContext(nc) as tc:
        _emit(nc, tc, P)
    nc.compile()
    nc.m = get_hw_module(nc.m)
    return nc


def _round_f32r(a):
    """Round fp32 array to fp32r (matmul operand precision) if AGG_FAST."""
    return a


def _shard_inputs(inputs):
    f = np.float32
    in_maps = []
    pad_w = []
    for m in range(M):
        w = np.zeros((DP, H), f)
        w[:D, :] = inputs[f"gc1_W{m}"]
        pad_w.append(np.ascontiguousarray(w))
    for c in range(NCORES):
        r0, r1 = c * S, (c + 1) * S
        im = {}
        for m in range(M):
            im[f"adjT{m}"] = np.ascontiguousarray(
                np.asarray(inputs[f"adj{m}"], f)[r0:r1, :].T)
            xp = np.zeros((DP, S), f)
            xp[:D, :] = np.asarray(inputs[f"x{m}"], f)[r0:r1, :].T
            im[f"xT{m}"] = xp
            im[f"gc1_W{m}"] = pad_w[m]
            im[f"gc1_b{m}"] = np.asarray(inputs[f"gc1_b{m}"], f).reshape(H, 1)
            im[f"gc2_W{m}"] = np.ascontiguousarray(np.asarray(inputs[f"gc2_W{m}"], f))
            im[f"gc2_b{m}"] = np.asarray(inputs[f"gc2_b{m}"], f).reshape(H, 1)
            im[f"gat_W{m}"] = np.ascontiguousarray(np.asarray(inputs[f"gat_W{m}"], f))
            im[f"gat_a{m}"] = np.ascontiguousarray(np.asarray(inputs[f"gat_a{m}"], f))
            im[f"spW1_{m}"] = np.ascontiguousarray(np.asarray(inputs[f"spW1_{m}"], f))
            im[f"spb1_{m}"] = np.asarray(inputs[f"spb1_{m}"], f).reshape(PH, 1)
            im[f"spW2_{m}"] = np.ascontiguousarray(np.asarray(inputs[f"spW2_{m}"], f))
            im[f"spb2_{m}"] = np.asarray(inputs[f"spb2_{m}"], f).reshape(Y, 1)
        im["jpW1"] = np.ascontiguousarray(np.asarray(inputs["jpW1"], f))
        im["jpb1"] = np.asarray(inputs["jpb1"], f).reshape(PH, 1)
        im["jpW2"] = np.ascontiguousarray(np.asarray(inputs["jpW2"], f))
        im["jpb2"] = np.asarray(inputs["jpb2"], f).reshape(Y, 1)
        im["maskT"] = np.ascontiguousarray(
            np.asarray(inputs["mask"], f)[r0:r1, :].T.reshape(1, M * S))
        in_maps.append(im)
    return in_maps


def run(inputs, trace=False):
    nc = _get_compiled()
    in_maps = _shard_inputs(inputs)
    res = run_bass_kernel_spmd(nc, in_maps, list(range(NCORES)), trace=trace)
    out = np.zeros((N, 3 * Y), np.float32)
    for c in range(NCORES):
        out[c * S:(c + 1) * S, :] = res.results[c]["outT"].T
    return out, res


def kernel(**inputs):
    out, _ = run(inputs)
    return out
